# revision 13
# baseline (speedup 1.0000x reference)
"""MDGRec GNN message-passing kernel for 8 Trainium2 NeuronCores.

Strategy (SPMD, one NEFF on 8 cores):
  - Nodes row-sharded: core m owns dst rows [m*18750, (m+1)*18750).
  - id and text features concatenated into 128-wide rows (512B) so every
    indirect gather moves a full 512B descriptor (DMA line-rate).
  - Layer tables (full [150000, 128]) built via on-device AllGather.
  - SpMM per layer: bulk dma_gather of h[edge_col] (int16 indices, node
    space split into 5 ranges of 30000), edge values applied on ScalarE,
    one-hot segment matrices built on VectorE from host-staged slot ids,
    segment-sum via PE matmuls accumulating in PSUM per 128-row group.
  - Fused epilogue (layer mean, tail amp, gate, blend) on device; final
    rows scattered into a per-core output shard.

Host side packs edges into a fixed template (identical instruction
stream across cores): G groups x 5 ranges x C_GR chunks of 128 edges.
"""

import numpy as np

import concourse.bass as bass
import concourse.bacc as bacc
import concourse.tile as tile
import concourse.mybir as mybir
from concourse import bass_utils, library_config
from concourse.masks import make_identity

# ---- problem constants (hardcoded per spec) ----
N_NODES = 150000
EMB_DIM = 64
TEXT_DIM = 384
NCORES = 8
SHARD = N_NODES // NCORES          # 18750
F = 2 * EMB_DIM                    # 128 concat feature width

# ---- template constants ----
G = 150                            # groups per core (>= ceil(18750/128))
S_G = 2                            # groups per supergroup
N_SG = G // S_G                    # 75
N_RANGE = 5
RANGE_SIZE = 30000                 # int16-safe gather range
C_GR = 7                           # chunks per (group, range)
CPG = N_RANGE * C_GR               # 35 chunks per group
C_SG = S_G * CPG                   # 70 chunks per supergroup
CALL_CH = S_G * C_GR               # 14 chunks per gather call
CALL_IDX = CALL_CH * 128           # 1792 idxs per gather call
CAP_R = C_GR * 128                 # 896 edge capacity per (group, range)
DUMMY = SHARD                      # dummy row in local shard buffers
PAD_SLOT = 999.0

_CACHE = {}


# ======================================================================
# device program
# ======================================================================

def _build(n_sg_run=N_SG, run_layers=(0, 1), do_collectives=True, dump=None):
    """dump: None | 'table0' | 'h1' — copies debug data into `out`."""
    fp32 = mybir.dt.float32
    bf16 = mybir.dt.bfloat16
    i32 = mybir.dt.int32
    i16 = mybir.dt.int16

    nc = bacc.Bacc("TRN2", target_bir_lowering=False, debug=False,
                   num_devices=NCORES)

    # inputs (per core)
    text_T = nc.dram_tensor("text_T", [TEXT_DIM, SHARD], fp32, kind="ExternalInput")
    id_shard = nc.dram_tensor("id_shard", [SHARD, EMB_DIM], fp32, kind="ExternalInput")
    gidx = nc.dram_tensor("gidx", [N_SG, N_RANGE, 128, CALL_IDX // 16], i16,
                          kind="ExternalInput")
    slot_a = nc.dram_tensor("slot_a", [N_SG, 128, C_SG], fp32, kind="ExternalInput")
    val_a = nc.dram_tensor("val_a", [N_SG, 128, C_SG], fp32, kind="ExternalInput")
    oidx = nc.dram_tensor("oidx", [G, 128], i32, kind="ExternalInput")
    tailf = nc.dram_tensor("tailf", [G, 128], fp32, kind="ExternalInput")
    w_text = nc.dram_tensor("w_text", [TEXT_DIM, EMB_DIM], fp32, kind="ExternalInput")
    b_text = nc.dram_tensor("b_text", [128, EMB_DIM], fp32, kind="ExternalInput")
    w_fuse = nc.dram_tensor("w_fuse", [F, EMB_DIM], fp32, kind="ExternalInput")
    b_fuse = nc.dram_tensor("b_fuse", [EMB_DIM, 1], fp32, kind="ExternalInput")
    iota_d = nc.dram_tensor("iota_d", [128, 128], fp32, kind="ExternalInput")
    zeros2 = nc.dram_tensor("zeros2", [2, F], fp32, kind="ExternalInput")

    out = nc.dram_tensor("out", [SHARD + 2, EMB_DIM], fp32, kind="ExternalOutput")

    # internal DRAM
    cat_shard = nc.dram_tensor("cat_shard", [SHARD + 2, F], fp32)
    h1_shard = nc.dram_tensor("h1_shard", [SHARD + 2, F], fp32)
    table0 = nc.dram_tensor("table0", [N_NODES, F], fp32, addr_space="Shared")
    table1 = nc.dram_tensor("table1", [N_NODES, F], fp32, addr_space="Shared")

    n_tiles = (SHARD + 127) // 128  # 147 row tiles for the projection

    with tile.TileContext(nc) as tc:
        nc.gpsimd.load_library(library_config.mlp)
        with (
            tc.tile_pool(name="const", bufs=1) as cpool,
            tc.tile_pool(name="sb", bufs=2) as sb,
            tc.tile_pool(name="xp", bufs=2) as xp,
            tc.tile_pool(name="psum", bufs=2, space="PSUM") as ps,
        ):
            # ---- constants ----
            iota_t = cpool.tile([128, 128], fp32, tag="iota")
            nc.sync.dma_start(iota_t[:], iota_d[:])
            ident = cpool.tile([128, 128], fp32, tag="ident")
            make_identity(nc, ident[:])
            wt_t = cpool.tile([128, 3 * EMB_DIM], fp32, tag="wt")
            for k in range(3):
                nc.sync.dma_start(wt_t[:, k * EMB_DIM:(k + 1) * EMB_DIM],
                                  w_text[k * 128:(k + 1) * 128, :])
            bt_t = cpool.tile([128, EMB_DIM], fp32, tag="bt")
            nc.sync.dma_start(bt_t[:], b_text[:])
            wf_t = cpool.tile([128, EMB_DIM], fp32, tag="wf")
            nc.sync.dma_start(wf_t[:], w_fuse[:])
            bf_t = cpool.tile([EMB_DIM, 1], fp32, tag="bf")
            nc.sync.dma_start(bf_t[:], b_fuse[:])

            # zero dummy rows of local shards
            zt = cpool.tile([2, F], fp32, tag="zt")
            nc.sync.dma_start(zt[:], zeros2[:])
            nc.sync.dma_start(cat_shard[SHARD:SHARD + 2, :], zt[:])
            nc.sync.dma_start(h1_shard[SHARD:SHARD + 2, :], zt[:])

            # ---- text projection + cat_shard assembly ----
            for i in range(n_tiles):
                r0 = i * 128
                rn = min(128, SHARD - r0)
                proj_ps = ps.tile([128, EMB_DIM], fp32, tag="mm")
                # load the three K-slices of text_T for this row tile
                tx3 = sb.tile([128, 3, 128], fp32, tag="tx3")
                for k in range(3):
                    nc.sync.dma_start(tx3[:, k, :rn],
                                      text_T[k * 128:(k + 1) * 128, r0:r0 + rn])
                for k in range(3):
                    nc.tensor.matmul(proj_ps[:rn, :], lhsT=tx3[:, k, :rn],
                                     rhs=wt_t[:, k * EMB_DIM:(k + 1) * EMB_DIM],
                                     start=(k == 0), stop=(k == 2))
                cat_t = sb.tile([128, F], fp32, tag="cat")
                nc.sync.dma_start(cat_t[:rn, 0:EMB_DIM], id_shard[r0:r0 + rn, :])
                nc.vector.tensor_tensor(out=cat_t[:rn, EMB_DIM:F],
                                        in0=proj_ps[:rn, :], in1=bt_t[:rn, :],
                                        op=mybir.AluOpType.add)
                nc.sync.dma_start(cat_shard[r0:r0 + rn, :], cat_t[:rn, :])

            # ---- AllGather h0 ----
            if do_collectives:
                nc.gpsimd.collective_compute(
                    "AllGather", mybir.AluOpType.bypass,
                    replica_groups=[list(range(NCORES))],
                    ins=[cat_shard[0:SHARD, :]],
                    outs=[table0[:]],
                )

            if dump == "table0":
                # copy own-shard rows of table0 into out (first 64 feats)
                for i in range(16):
                    dbg = sb.tile([128, EMB_DIM], fp32, tag="dbg")
                    nc.sync.dma_start(dbg[:], table0[i * 128:(i + 1) * 128, 0:EMB_DIM])
                    nc.sync.dma_start(out[i * 128:(i + 1) * 128, :], dbg[:])

            # ---- SpMM layers ----
            import os
            l1_stage = os.environ.get("L1_STAGE", "full")
            for layer in run_layers:
                table = table0 if layer == 0 else table1
                for sg in range(n_sg_run):
                    slot_t = sb.tile([128, C_SG], fp32, tag="slot")
                    nc.sync.dma_start(slot_t[:], slot_a[sg, :, :])
                    val_t = sb.tile([128, C_SG], fp32, tag="val")
                    nc.sync.dma_start(val_t[:], val_a[sg, :, :])

                    X = xp.tile([128, C_SG, F], fp32, tag="X")
                    for r in range(int(os.environ.get("L1_RANGES", N_RANGE))):
                        gi = sb.tile([128, CALL_IDX // 16], i16, tag="gi")
                        nc.sync.dma_start(gi[:], gidx[sg, r, :, :])
                        nc.gpsimd.dma_gather(
                            X[:, r * CALL_CH:(r + 1) * CALL_CH, :],
                            table[r * RANGE_SIZE:(r + 1) * RANGE_SIZE, :],
                            gi[:], CALL_IDX, CALL_IDX, F,
                            single_packet=False)

                    if l1_stage == "gather":
                        dbg = sb.tile([128, F], fp32, tag="res")
                        nc.vector.tensor_copy(dbg[:], X[:, 0, :])
                        nc.sync.dma_start(out[sg * 128:(sg + 1) * 128, :],
                                          dbg[:, 0:EMB_DIM])
                        continue

                    S_t = xp.tile([128, C_SG, 128], bf16, tag="S")
                    nc.vector.tensor_tensor(
                        out=S_t[:],
                        in0=slot_t[:].rearrange("p (c o) -> p c o", o=1)
                            .to_broadcast([128, C_SG, 128]),
                        in1=iota_t[:].rearrange("p (o j) -> p o j", o=1)
                            .to_broadcast([128, C_SG, 128]),
                        op=mybir.AluOpType.is_equal,
                    )
                    if l1_stage == "onehot":
                        dbg = sb.tile([128, F], fp32, tag="res")
                        nc.vector.tensor_copy(dbg[:], S_t[:, 0, :])
                        nc.sync.dma_start(out[sg * 128:(sg + 1) * 128, :],
                                          dbg[:, 0:EMB_DIM])
                        continue

                    Xs = xp.tile([128, C_SG, F], bf16, tag="Xs")
                    for ci in range(C_SG):
                        nc.scalar.activation(
                            Xs[:, ci, :], X[:, ci, :],
                            mybir.ActivationFunctionType.Copy,
                            scale=val_t[:, ci:ci + 1])
                    if l1_stage == "scale":
                        dbg = sb.tile([128, F], fp32, tag="res")
                        nc.vector.tensor_copy(dbg[:], Xs[:, 0, :])
                        nc.sync.dma_start(out[sg * 128:(sg + 1) * 128, :],
                                          dbg[:, 0:EMB_DIM])
                        continue

                    for s in range(S_G):
                        g = sg * S_G + s
                        acc = ps.tile([128, F], fp32, tag="mm")
                        chunks = [r * CALL_CH + s * C_GR + c
                                  for r in range(N_RANGE) for c in range(C_GR)]
                        for j, ci in enumerate(chunks):
                            nc.tensor.matmul(acc[:], lhsT=S_t[:, ci, :],
                                             rhs=Xs[:, ci, :],
                                             start=(j == 0), stop=(j == CPG - 1))
                        oix = sb.tile([128, 1], i32, tag="oix")
                        nc.sync.dma_start(oix[:], oidx[g, :, None])

                        if layer == 0:
                            res = sb.tile([128, F], fp32, tag="res")
                            nc.vector.tensor_copy(res[:], acc[:])
                            if l1_stage == "mm":
                                nc.sync.dma_start(
                                    out[g * 128:(g + 1) * 128, :],
                                    res[:, 0:EMB_DIM])
                                continue
                            nc.gpsimd.indirect_dma_start(
                                out=h1_shard[:], out_offset=bass.IndirectOffsetOnAxis(
                                    ap=oix[:, :1], axis=0),
                                in_=res[:], in_offset=None)
                        else:
                            # fused epilogue for this group's rows
                            h0_t = sb.tile([128, F], fp32, tag="h0")
                            nc.gpsimd.indirect_dma_start(
                                out=h0_t[:], out_offset=None,
                                in_=cat_shard[:], in_offset=bass.IndirectOffsetOnAxis(
                                    ap=oix[:, :1], axis=0))
                            h1_t = sb.tile([128, F], fp32, tag="h1")
                            nc.gpsimd.indirect_dma_start(
                                out=h1_t[:], out_offset=None,
                                in_=h1_shard[:], in_offset=bass.IndirectOffsetOnAxis(
                                    ap=oix[:, :1], axis=0))
                            tf_t = sb.tile([128, 1], fp32, tag="tf")
                            nc.sync.dma_start(tf_t[:], tailf[g, :, None])

                            fsum = sb.tile([128, F], fp32, tag="fsum")
                            nc.vector.tensor_tensor(out=fsum[:], in0=h0_t[:],
                                                    in1=h1_t[:],
                                                    op=mybir.AluOpType.add)
                            nc.vector.tensor_tensor(out=fsum[:], in0=fsum[:],
                                                    in1=acc[:],
                                                    op=mybir.AluOpType.add)
                            # id half * 1/3, text half * tailf (amp/3 folded)
                            nc.vector.tensor_scalar_mul(
                                fsum[:, 0:EMB_DIM], fsum[:, 0:EMB_DIM], 1.0 / 3.0)
                            nc.vector.tensor_scalar_mul(
                                fsum[:, EMB_DIM:F], fsum[:, EMB_DIM:F], tf_t[:, :1])

                            tp = ps.tile([128, 128], fp32, tag="tp")
                            nc.tensor.transpose(out=tp[:], in_=fsum[:],
                                                identity=ident[:])
                            ft = sb.tile([128, 128], fp32, tag="ft")
                            nc.vector.tensor_copy(ft[:], tp[:])

                            gp = ps.tile([EMB_DIM, 128], fp32, tag="gp")
                            nc.tensor.matmul(gp[:], lhsT=wf_t[:], rhs=ft[:],
                                             start=True, stop=True)
                            gate_T = sb.tile([EMB_DIM, 128], fp32, tag="gateT")
                            nc.scalar.activation(gate_T[:], gp[:],
                                                 mybir.ActivationFunctionType.Sigmoid,
                                                 bias=bf_t[:, :1])
                            g2 = ps.tile([128, EMB_DIM], fp32, tag="g2")
                            nc.tensor.transpose(out=g2[:], in_=gate_T[:],
                                                identity=ident[0:EMB_DIM, 0:EMB_DIM])
                            gate = sb.tile([128, EMB_DIM], fp32, tag="gate")
                            nc.vector.tensor_copy(gate[:], g2[:])

                            dif = sb.tile([128, EMB_DIM], fp32, tag="dif")
                            nc.vector.tensor_tensor(out=dif[:],
                                                    in0=fsum[:, 0:EMB_DIM],
                                                    in1=fsum[:, EMB_DIM:F],
                                                    op=mybir.AluOpType.subtract)
                            nc.vector.tensor_tensor(out=dif[:], in0=dif[:],
                                                    in1=gate[:],
                                                    op=mybir.AluOpType.mult)
                            fused = sb.tile([128, EMB_DIM], fp32, tag="fused")
                            nc.vector.tensor_tensor(out=fused[:],
                                                    in0=fsum[:, EMB_DIM:F],
                                                    in1=dif[:],
                                                    op=mybir.AluOpType.add)
                            nc.gpsimd.indirect_dma_start(
                                out=out[:], out_offset=bass.IndirectOffsetOnAxis(
                                    ap=oix[:, :1], axis=0),
                                in_=fused[:], in_offset=None)

                if layer == 0 and do_collectives and 1 in run_layers:
                    nc.gpsimd.collective_compute(
                        "AllGather", mybir.AluOpType.bypass,
                        replica_groups=[list(range(NCORES))],
                        ins=[h1_shard[0:SHARD, :]],
                        outs=[table1[:]],
                    )

            if dump == "h1":
                for i in range(16):
                    dbg = sb.tile([128, EMB_DIM], fp32, tag="dbg")
                    nc.sync.dma_start(dbg[:], h1_shard[i * 128:(i + 1) * 128, 0:EMB_DIM])
                    nc.sync.dma_start(out[i * 128:(i + 1) * 128, :], dbg[:])

    nc.compile()
    return nc


# ======================================================================
# host preprocessing
# ======================================================================

def _pack_groups(er, deg):
    """Assign each local row to a group s.t. per-(group, range) edge counts
    fit CAP_R and <=128 rows per group. deg: [SHARD, 5] per-range degrees."""
    import heapq
    total = deg.sum(1)
    order = np.argsort(-total, kind="stable")
    grp_of_row = np.full(SHARD, -1, np.int32)
    loads = np.zeros((G, N_RANGE), np.int64)
    nrows = np.zeros(G, np.int64)
    heap = [(0, g) for g in range(G)]
    heapq.heapify(heap)
    for row in order:
        d = deg[row]
        popped = []
        placed = False
        while heap:
            load, g = heapq.heappop(heap)
            if nrows[g] < 128 and np.all(loads[g] + d <= CAP_R):
                grp_of_row[row] = g
                loads[g] += d
                nrows[g] += 1
                heapq.heappush(heap, (int(loads[g].sum()), g))
                placed = True
                break
            popped.append((load, g))
        for item in popped:
            heapq.heappush(heap, item)
        if not placed:
            raise RuntimeError("group packing failed; raise G or C_GR")
    return grp_of_row, nrows


def _preprocess_core(m, edge_row, edge_col, edge_val, tail_mask, amp):
    lo = m * SHARD
    sel = (edge_row >= lo) & (edge_row < lo + SHARD)
    er = (edge_row[sel] - lo).astype(np.int64)
    ec = edge_col[sel].astype(np.int64)
    ev = edge_val[sel].astype(np.float32)

    rng_id = ec // RANGE_SIZE
    ec_loc = (ec - rng_id * RANGE_SIZE).astype(np.int64)

    # per-(row, range) degrees
    deg = np.zeros((SHARD, N_RANGE), np.int64)
    np.add.at(deg, (er, rng_id), 1)

    grp_of_row, nrows = _pack_groups(er, deg)

    # slot of each row within its group (stable order by row id)
    order_rows = np.lexsort((np.arange(SHARD), grp_of_row))
    slot_of_row = np.empty(SHARD, np.int64)
    # vectorized: rows sorted by group; slot = running index within group
    sorted_g = grp_of_row[order_rows]
    starts = np.searchsorted(sorted_g, np.arange(G))
    slot_sorted = np.arange(SHARD) - starts[sorted_g]
    slot_of_row[order_rows] = slot_sorted

    members = np.full((G, 128), DUMMY, np.int64)
    members[grp_of_row[order_rows], slot_sorted] = order_rows

    # ---- edge template fill ----
    g_e = grp_of_row[er]
    bucket = g_e * N_RANGE + rng_id
    eorder = np.argsort(bucket, kind="stable")
    b_sorted = bucket[eorder]
    cnt = np.bincount(b_sorted, minlength=G * N_RANGE)
    if cnt.max() > CAP_R:
        raise RuntimeError("bucket overflow despite packing")
    off = np.zeros(G * N_RANGE + 1, np.int64)
    np.cumsum(cnt, out=off[1:])
    pos = np.arange(len(eorder)) - off[b_sorted]

    e_g = g_e[eorder]
    e_r = rng_id[eorder]
    e_sg = e_g // S_G
    e_s = e_g % S_G
    e_c = pos // 128
    e_p = pos % 128
    e_ci = e_r * CALL_CH + e_s * C_GR + e_c

    slot_a = np.full((N_SG, 128, C_SG), PAD_SLOT, np.float32)
    val_a = np.zeros((N_SG, 128, C_SG), np.float32)
    lin = (e_sg * 128 + e_p) * C_SG + e_ci
    slot_a.reshape(-1)[lin] = slot_of_row[er[eorder]].astype(np.float32)
    val_a.reshape(-1)[lin] = ev[eorder]

    # gather indices, wrapped in 16 partitions, replicated x8
    gidx16 = np.zeros((N_SG, N_RANGE, 16, CALL_IDX // 16), np.int16)
    e_k = e_s * C_GR + e_c          # chunk within call (0..13)
    q = e_k * 128 + e_p             # idx position within call
    lin2 = ((e_sg * N_RANGE + e_r) * 16 + (q % 16)) * (CALL_IDX // 16) + (q // 16)
    gidx16.reshape(-1)[lin2] = ec_loc[eorder].astype(np.int16)
    gidx = np.tile(gidx16, (1, 1, 8, 1))

    oidx = members.astype(np.int32)
    # tail factor per member (amp applied to text half, divided by 3)
    tmask = tail_mask[lo:lo + SHARD].astype(np.float32)
    tf_row = np.where(tmask > 0, amp, 1.0).astype(np.float32) / 3.0
    tf_row = np.concatenate([tf_row, np.full(2, 1.0 / 3.0, np.float32)])
    tailf = tf_row[np.minimum(members, SHARD)].astype(np.float32)

    return {
        "gidx": gidx, "slot_a": slot_a, "val_a": val_a,
        "oidx": oidx, "tailf": tailf,
    }


def kernel(text_feats, edge_row, edge_col, edge_val, tail_mask, user_emb,
           item_emb, W_text, b_text, W_fuse, b_fuse, tail_amp):
    text_feats = np.asarray(text_feats, np.float32)
    edge_row = np.asarray(edge_row).astype(np.int64)
    edge_col = np.asarray(edge_col).astype(np.int64)
    edge_val = np.asarray(edge_val, np.float32)
    tail_mask = np.asarray(tail_mask).astype(bool)
    user_emb = np.asarray(user_emb, np.float32)
    item_emb = np.asarray(item_emb, np.float32)
    W_text = np.asarray(W_text, np.float32)
    b_text = np.asarray(b_text, np.float32)
    W_fuse = np.asarray(W_fuse, np.float32)
    b_fuse = np.asarray(b_fuse, np.float32)
    amp = float(1.0 + 1.0 / (1.0 + np.exp(-np.float64(np.asarray(tail_amp)))))

    emb_id = np.concatenate([user_emb, item_emb], axis=0)  # [N, 64]

    if "nc" not in _CACHE:
        _CACHE["nc"] = _build()
    nc = _CACHE["nc"]

    iota = np.tile(np.arange(128, dtype=np.float32)[None, :], (128, 1))
    b_text_rep = np.tile(b_text[None, :], (128, 1)).astype(np.float32)
    b_fuse_col = b_fuse[:, None].astype(np.float32)
    zeros2 = np.zeros((2, F), np.float32)

    in_maps = []
    for m in range(NCORES):
        pre = _preprocess_core(m, edge_row, edge_col, edge_val, tail_mask, amp)
        lo = m * SHARD
        in_maps.append({
            "text_T": np.ascontiguousarray(text_feats[lo:lo + SHARD].T),
            "id_shard": np.ascontiguousarray(emb_id[lo:lo + SHARD]),
            "gidx": pre["gidx"], "slot_a": pre["slot_a"], "val_a": pre["val_a"],
            "oidx": pre["oidx"], "tailf": pre["tailf"],
            "w_text": W_text, "b_text": b_text_rep,
            "w_fuse": W_fuse, "b_fuse": b_fuse_col,
            "iota_d": iota, "zeros2": zeros2,
        })

    global _LAST_IN_MAPS
    _LAST_IN_MAPS = in_maps
    res = bass_utils.run_bass_kernel_spmd(nc, in_maps, core_ids=list(range(NCORES)))
    out = np.concatenate(
        [res.results[m]["out"][0:SHARD] for m in range(NCORES)], axis=0)
    return out.astype(np.float32)
